# revision 1
# baseline (speedup 1.0000x reference)
"""ChebNet (K=3, 7 ChebConv layers) on 8 Trainium2 NeuronCores.

Strategy
--------
Nodes are partitioned contiguously across the 8 cores (12500/core); each
core owns the edges whose dst falls in its shard.  Each ChebConv layer
    relu(cat(X0,X1,X2) @ W + b),  X1 = -A_hat X,  X2 = -2 A_hat X1 - X0
is rewritten (exactly) as
    relu(H @ Wa + A_hat (H @ Wb + A_hat (H @ Wc)) + b)
with Wa = W0' - W2', Wb = -W1', Wc = 2 W2'  (W = [W0'; W1'; W2']),
so the SpMM operates on post-matmul activations and the layer needs
exactly two halo exchanges (AllGather of the f32, dinv-prescaled shard).
Everything stays f32: the network's output scale is ~250x smaller than
its intermediates, so bf16 anywhere in the message path amplifies to
~100% relative error.

SpMM on-device: edges are bucketed host-side by (dst-chunk of 128,
src-window of 25000 [int16 gather addressing]) into a static, padded
schedule shared by all 8 cores (one SPMD program).  Messages are fetched
with dma_gather (512B f32 rows; <=1024 idxs per call — larger wedges the
device), segment-summed per dst-chunk with one-hot matmuls accumulating
in PSUM (one-hots built on the DVE from per-edge slot ids vs an iota;
padded slots get slot=255 and multiply to zero), then scaled by
dinv[dst] and consumed.  Za/Zb spill to DRAM to keep the f32 message
tiles resident in SBUF.
"""

import numpy as np
import ml_dtypes

import concourse.bass as bass
import concourse.bacc as bacc
import concourse.mybir as mybir
import concourse.tile as tile
import concourse.bass_utils as bass_utils

P = 128
F32 = mybir.dt.float32
BF16 = mybir.dt.bfloat16
I16 = mybir.dt.int16


class Cfg:
    def __init__(self, N, ncores, D, OUT, num_hid, wsz, G):
        self.N = N
        self.NCORES = ncores
        self.SHARD = N // ncores
        self.NCHUNK = (self.SHARD + P - 1) // P
        self.NPAD = self.NCHUNK * P          # padded shard rows
        self.D = D                            # hidden width (=IN)
        self.OUT = OUT
        self.NL = num_hid + 2                 # total ChebConv layers
        self.WSZ = wsz                        # src window size (int16 range)
        self.NW = (N + wsz - 1) // wsz
        self.G = G                            # chunks per gather group
        assert self.NCHUNK % G == 0
        self.NG = self.NCHUNK // G
        # CAPW filled in by prep (data dependent, 128-aligned)
        self.CAPW = None
        self.NBW = None                       # blocks per (chunk, window)
        self.NB = None                        # blocks per chunk
        self.RW = None                        # slots per gather call
        self.RB = None                        # blocks per gather call


def make_cfg_full():
    return Cfg(N=100000, ncores=8, D=128, OUT=40, num_hid=5, wsz=25000, G=7)


def prep(inputs, cfg):
    """Host-side graph preprocessing -> per-core input maps."""
    src = np.asarray(inputs["src"]).astype(np.int64)
    dst = np.asarray(inputs["dst"]).astype(np.int64)
    feat = np.asarray(inputs["features"], dtype=np.float32)
    N, C = cfg.N, cfg.NCORES

    deg = np.bincount(dst, minlength=N).astype(np.float32)
    dinv = np.clip(deg, 1.0, None) ** -0.5

    core = dst // cfg.SHARD
    loc = dst % cfg.SHARD                   # row within the owning shard
    chunk = loc // P
    lane = loc % P                          # slot id within chunk
    win = src // cfg.WSZ
    idx16 = (src % cfg.WSZ).astype(np.int16)

    # per (core, chunk, window) edge lists
    key = ((core * cfg.NCHUNK + chunk) * cfg.NW + win).astype(np.int64)
    order = np.argsort(key, kind="stable")
    counts = np.bincount(key, minlength=C * cfg.NCHUNK * cfg.NW)
    counts = counts.reshape(C, cfg.NCHUNK, cfg.NW)
    capw = int(counts.max())
    cfg.CAPW = ((capw + P - 1) // P) * P
    cfg.NBW = cfg.CAPW // P
    cfg.NB = cfg.NBW * cfg.NW
    cfg.RW = cfg.G * cfg.CAPW
    cfg.RB = cfg.RW // P

    src_sorted = idx16[order]
    lane_sorted = lane[order].astype(np.int32)
    starts = np.zeros(C * cfg.NCHUNK * cfg.NW + 1, np.int64)
    np.cumsum(counts.ravel(), out=starts[1:])

    # layer weights -> [NL, D, 3*fout] slabs (fp32)
    NL = cfg.NL
    wabc = np.zeros((NL, cfg.D, 3 * cfg.D), np.float32)
    bbc = np.zeros((NL, P, cfg.D), np.float32)

    def pack(Wfull, b, li, fout):
        Wfull = np.asarray(Wfull, dtype=np.float32)
        d = Wfull.shape[0] // 3
        W0, W1, W2 = Wfull[:d], Wfull[d:2 * d], Wfull[2 * d:]
        Wa, Wb, Wc = W0 - W2, -W1, 2.0 * W2
        wabc[li, :, 0 * fout:1 * fout] = Wa
        wabc[li, :, 1 * fout:2 * fout] = Wb
        wabc[li, :, 2 * fout:3 * fout] = Wc
        bbc[li, :, :fout] = np.tile(np.asarray(b, dtype=np.float32)[None, :], (P, 1))

    pack(inputs["W0"], inputs["b0"], 0, cfg.D)
    for i in range(NL - 2):
        pack(np.asarray(inputs["Wh"])[i], np.asarray(inputs["bh"])[i], 1 + i, cfg.D)
    pack(inputs["Wl"], inputs["bl"], NL - 1, cfg.OUT)

    iota_rep = np.tile(
        np.arange(P, dtype=np.float32)[None, None, :], (P, cfg.NB, 1))
    ident = np.eye(P, dtype=np.float32)

    in_maps = []
    for c in range(C):
        tot = cfg.NCHUNK * cfg.NB * P        # slots per spmm
        idxs = np.zeros(tot, np.int16)
        slots_cols = np.full((P, cfg.NCHUNK * cfg.NB), 255.0, np.float32)
        pos = 0
        for g in range(cfg.NG):
            for w in range(cfg.NW):
                for i in range(cfg.G):
                    q = g * cfg.G + i
                    k = (c * cfg.NCHUNK + q) * cfg.NW + w
                    s, e = starts[k], starts[k + 1]
                    n = e - s
                    idxs[pos:pos + n] = src_sorted[s:e]
                    # block-columns for this (q, w): cols q*NB + w*NBW + j
                    seg_sl = lane_sorted[s:e]
                    for j in range(cfg.NBW):
                        col = q * cfg.NB + w * cfg.NBW + j
                        a, b_ = j * P, min((j + 1) * P, n)
                        if a < n:
                            slots_cols[:b_ - a, col] = seg_sl[a:b_]
                    pos += cfg.CAPW
        assert pos == tot
        # wrapped idx layout [128, tot/16]
        wr = idxs.reshape(tot // 16, 16).T
        idxs_w = np.tile(wr, (8, 1)).copy()

        sh0 = c * cfg.SHARD
        fpad = np.zeros((cfg.NPAD, cfg.D), np.float32)
        fpad[:cfg.SHARD] = feat[sh0:sh0 + cfg.SHARD]
        dv = dinv[sh0:sh0 + cfg.SHARD]
        dm = np.ones((P, cfg.NCHUNK), np.float32)
        for q in range(cfg.NCHUNK):
            r = min(P, cfg.SHARD - q * P)
            dm[:r, q] = dv[q * P:q * P + r]
        in_maps.append(dict(
            feat=fpad,
            idxs=idxs_w,
            slots=slots_cols,
            dinvc=dm,
            dinv2c=(dm * dm).astype(np.float32),
            wabc=wabc,
            bbc=bbc,
            iotarep=iota_rep,
            ident=ident,
        ))
    return in_maps


def build(nc, cfg):
    NL, D, OUT = cfg.NL, cfg.D, cfg.OUT
    NCH, NB, NBW, NW, G, NG = cfg.NCHUNK, cfg.NB, cfg.NBW, cfg.NW, cfg.G, cfg.NG
    RW, RB = cfg.RW, cfg.RB
    TOT16 = NCH * NB * P // 16

    feat_in = nc.dram_tensor("feat", [cfg.NPAD, D], F32, kind="ExternalInput")
    idxs_in = nc.dram_tensor("idxs", [P, TOT16], I16, kind="ExternalInput")
    slots_in = nc.dram_tensor("slots", [P, NCH * NB], F32, kind="ExternalInput")
    dinv_in = nc.dram_tensor("dinvc", [P, NCH], F32, kind="ExternalInput")
    dinv2_in = nc.dram_tensor("dinv2c", [P, NCH], F32, kind="ExternalInput")
    wabc_in = nc.dram_tensor("wabc", [NL, D, 3 * D], F32, kind="ExternalInput")
    bbc_in = nc.dram_tensor("bbc", [NL, P, D], F32, kind="ExternalInput")
    iota_in = nc.dram_tensor("iotarep", [P, NB, P], F32, kind="ExternalInput")
    ident_in = nc.dram_tensor("ident", [P, P], F32, kind="ExternalInput")
    out_dram = nc.dram_tensor("out", [cfg.SHARD, OUT], F32, kind="ExternalOutput")

    with tile.TileContext(nc) as tc:
        with (
            tc.tile_pool(name="persist", bufs=1) as pp,
            tc.tile_pool(name="work", bufs=2) as wk,
            tc.tile_pool(name="msgp", bufs=5) as mp,
            tc.tile_pool(name="idxp", bufs=3) as ip,
            tc.tile_pool(name="psum", bufs=2, space="PSUM") as ps,
            tc.tile_pool(name="praw", bufs=4, space="PSUM") as pr,
            tc.tile_pool(name="dram", bufs=1, space="DRAM") as dr,
        ):
            # persistent state (Za/Zb spill to DRAM; SBUF holds H + msg tiles)
            H = pp.tile([P, NCH, D], F32, tag="H")
            iota_t = pp.tile([P, NB, P], F32, tag="iota")
            ident_t = pp.tile([P, P], F32, tag="ident")
            dinv_t = pp.tile([P, NCH], F32, tag="dinv")
            dinv2_t = pp.tile([P, NCH], F32, tag="dinv2")
            nc.sync.dma_start(iota_t[:], iota_in[:, :, :])
            nc.sync.dma_start(ident_t[:], ident_in[:, :])
            nc.sync.dma_start(dinv_t[:], dinv_in[:, :])
            nc.sync.dma_start(dinv2_t[:], dinv2_in[:, :])
            nc.sync.dma_start(
                H[:], feat_in[:, :].rearrange("(q p) f -> p q f", p=P))

            gin1 = dr.tile([cfg.SHARD, D], F32, tag="gin1")
            gout1 = dr.tile([cfg.N, D], F32, tag="gout1")
            gin2 = dr.tile([cfg.SHARD, D], F32, tag="gin2")
            gout2 = dr.tile([cfg.N, D], F32, tag="gout2")
            za_dram = dr.tile([cfg.NPAD, D], F32, tag="za")
            zb_dram = dr.tile([cfg.NPAD, D], F32, tag="zb")

            def ag(gin, gout):
                nc.gpsimd.collective_compute(
                    "AllGather",
                    mybir.AluOpType.bypass,
                    replica_groups=[list(range(cfg.NCORES))],
                    ins=[gin.opt()],
                    outs=[gout.opt()],
                )

            def spmm(gout, fout, consume):
                """Gather from gout, segment-sum per chunk, call consume(q, praw_ap)."""
                # dma_gather wedges the device above ~1k idxs/call: sub-gather
                # per chunk (CAPW slots) out of the shared per-(g,w) idx tile.
                assert cfg.CAPW <= 1024
                for g in range(NG):
                    mg = []
                    for w in range(NW):
                        r = g * NW + w
                        it = ip.tile([P, RW // 16], I16, tag="idx")
                        nc.sync.dma_start(
                            it[:], idxs_in[:, r * (RW // 16):(r + 1) * (RW // 16)])
                        m = mp.tile([P, RB, D], F32, tag="msg")
                        c16 = cfg.CAPW // 16
                        for i in range(G):
                            nc.gpsimd.dma_gather(
                                m[:, i * NBW:(i + 1) * NBW, :],
                                gout[w * cfg.WSZ:min((w + 1) * cfg.WSZ, cfg.N), :],
                                it[:, i * c16:(i + 1) * c16],
                                cfg.CAPW,
                                cfg.CAPW,
                                D,
                            )
                        mg.append(m)
                    st = wk.tile([P, G * NB], F32, tag="slots")
                    nc.sync.dma_start(
                        st[:], slots_in[:, g * G * NB:(g + 1) * G * NB])
                    for i in range(G):
                        q = g * G + i
                        oh = wk.tile([P, NB, P], F32, tag="oh")
                        nc.vector.tensor_tensor(
                            out=oh[:],
                            in0=iota_t[:],
                            in1=st[:, i * NB:(i + 1) * NB].to_broadcast([P, NB, P]),
                            op=mybir.AluOpType.is_equal,
                        )
                        acc = pr.tile([P, D], F32, tag="praw")
                        nb = 0
                        for w in range(NW):
                            for j in range(NBW):
                                nc.tensor.matmul(
                                    acc[:, :fout],
                                    lhsT=oh[:, w * NBW + j, :],
                                    rhs=mg[w][:, i * NBW + j, :fout],
                                    start=(nb == 0),
                                    stop=(nb == NB - 1),
                                )
                                nb += 1
                        consume(q, acc)

            for li in range(NL):
                fout = OUT if li == NL - 1 else D
                wt = wk.tile([P, 3 * D], F32, tag="wt")
                nc.sync.dma_start(wt[:], wabc_in[li, :, :])
                bb = wk.tile([P, D], F32, tag="bbc")
                nc.sync.dma_start(bb[:], bbc_in[li, :, :])

                # Z phase: Za' = H Wa + b; ZbD = dinv*(H Wb); U2 = dinv*(H Wc) -> gin1
                for q in range(NCH):
                    rows = min(P, cfg.SHARD - q * P)
                    tp = ps.tile([P, P], F32, tag="tp")
                    nc.tensor.transpose(tp[:], H[:, q, :], ident_t[:])
                    ht = wk.tile([P, P], F32, tag="ht")
                    nc.vector.tensor_copy(ht[:], tp[:])
                    pz = ps.tile([P, 3 * D], F32, tag="pz")
                    nc.tensor.matmul(pz[:, :3 * fout], lhsT=ht[:], rhs=wt[:, :3 * fout],
                                     start=True, stop=True)
                    za_t = wk.tile([P, P], F32, tag="za_t")
                    nc.vector.tensor_tensor(
                        out=za_t[:, :fout], in0=pz[:, 0:fout], in1=bb[:, :fout],
                        op=mybir.AluOpType.add)
                    nc.sync.dma_start(za_dram[q * P:(q + 1) * P, :fout],
                                      za_t[:, :fout])
                    zb_t = wk.tile([P, P], F32, tag="zb_t")
                    nc.vector.tensor_scalar(
                        out=zb_t[:, :fout], in0=pz[:, fout:2 * fout],
                        scalar1=dinv_t[:, q:q + 1], scalar2=None,
                        op0=mybir.AluOpType.mult)
                    nc.sync.dma_start(zb_dram[q * P:(q + 1) * P, :fout],
                                      zb_t[:, :fout])
                    u2 = wk.tile([P, P], F32, tag="u2")
                    nc.vector.tensor_scalar(
                        out=u2[:, :fout], in0=pz[:, 2 * fout:3 * fout],
                        scalar1=dinv_t[:, q:q + 1], scalar2=None,
                        op0=mybir.AluOpType.mult)
                    nc.sync.dma_start(gin1[q * P:q * P + rows, :fout],
                                      u2[:rows, :fout])

                ag(gin1, gout1)

                def consume1(q, acc, fout=fout):
                    rows = min(P, cfg.SHARD - q * P)
                    zb_l = wk.tile([P, P], F32, tag="zb_l")
                    nc.sync.dma_start(zb_l[:, :fout],
                                      zb_dram[q * P:(q + 1) * P, :fout])
                    t1 = wk.tile([P, P], F32, tag="t1")
                    nc.vector.tensor_scalar(
                        out=t1[:, :fout], in0=acc[:, :fout],
                        scalar1=dinv2_t[:, q:q + 1], scalar2=None,
                        op0=mybir.AluOpType.mult)
                    t2 = wk.tile([P, P], F32, tag="t2")
                    nc.vector.tensor_tensor(
                        out=t2[:, :fout], in0=t1[:, :fout], in1=zb_l[:, :fout],
                        op=mybir.AluOpType.add)
                    nc.sync.dma_start(gin2[q * P:q * P + rows, :fout],
                                      t2[:rows, :fout])

                spmm(gout1, fout, consume1)
                ag(gin2, gout2)

                if li < NL - 1:
                    def consume2(q, acc, fout=fout):
                        za_l = wk.tile([P, P], F32, tag="za_l")
                        nc.sync.dma_start(za_l[:, :fout],
                                          za_dram[q * P:(q + 1) * P, :fout])
                        t1 = wk.tile([P, P], F32, tag="t1")
                        nc.vector.tensor_scalar(
                            out=t1[:, :fout], in0=acc[:, :fout],
                            scalar1=dinv_t[:, q:q + 1], scalar2=None,
                            op0=mybir.AluOpType.mult)
                        t3 = wk.tile([P, P], F32, tag="t3")
                        nc.vector.tensor_tensor(
                            out=t3[:, :fout], in0=t1[:, :fout], in1=za_l[:, :fout],
                            op=mybir.AluOpType.add)
                        nc.scalar.activation(
                            H[:, q, :fout], t3[:, :fout],
                            mybir.ActivationFunctionType.Relu)
                else:
                    def consume2(q, acc, fout=fout):
                        rows = min(P, cfg.SHARD - q * P)
                        za_l = wk.tile([P, P], F32, tag="za_l")
                        nc.sync.dma_start(za_l[:, :fout],
                                          za_dram[q * P:(q + 1) * P, :fout])
                        t1 = wk.tile([P, P], F32, tag="t1")
                        nc.vector.tensor_scalar(
                            out=t1[:, :fout], in0=acc[:, :fout],
                            scalar1=dinv_t[:, q:q + 1], scalar2=None,
                            op0=mybir.AluOpType.mult)
                        t3 = wk.tile([P, P], F32, tag="t3")
                        nc.vector.tensor_tensor(
                            out=t3[:, :fout], in0=t1[:, :fout], in1=za_l[:, :fout],
                            op=mybir.AluOpType.add)
                        ho = wk.tile([P, P], F32, tag="ho")
                        nc.scalar.activation(
                            ho[:, :fout], t3[:, :fout],
                            mybir.ActivationFunctionType.Relu)
                        nc.sync.dma_start(out_dram[q * P:q * P + rows, :fout],
                                          ho[:rows, :fout])

                spmm(gout2, fout, consume2)
    return nc


def run(inputs, cfg, trace=False):
    in_maps = prep(inputs, cfg)
    nc = bacc.Bacc("TRN2", target_bir_lowering=False, debug=False,
                   num_devices=cfg.NCORES)
    build(nc, cfg)
    nc.compile()
    res = bass_utils.run_bass_kernel_spmd(
        nc, in_maps, core_ids=list(range(cfg.NCORES)), trace=trace)
    out = np.concatenate([res.results[c]["out"] for c in range(cfg.NCORES)],
                         axis=0)
    return out[:cfg.N], res


def kernel(**inputs) -> np.ndarray:
    cfg = make_cfg_full()
    out, _ = run(inputs, cfg)
    return out.astype(np.float32)



# revision 6
# speedup vs baseline: 1.0278x; 1.0278x over previous
"""ChebNet (K=3, 7 ChebConv layers) on 8 Trainium2 NeuronCores.

Strategy
--------
Nodes are partitioned contiguously across the 8 cores (12500/core); each
core owns the edges whose dst falls in its shard.  Each ChebConv layer
    relu(cat(X0,X1,X2) @ W + b),  X1 = -A_hat X,  X2 = -2 A_hat X1 - X0
is rewritten (exactly) as
    relu(H @ Wa + A_hat (H @ Wb + A_hat (H @ Wc)) + b)
with Wa = W0' - W2', Wb = -W1', Wc = 2 W2'  (W = [W0'; W1'; W2']),
so the SpMM operates on post-matmul activations and the layer needs
exactly two halo exchanges (AllGather of the f32, dinv-prescaled shard).
The message path (AllGather payload + gathered messages) is bf16 with
f32 PSUM accumulation: the payload is rounded to bf16 once per
exchange, and the one-hot segment-sum matmul accumulates those bf16
messages exactly in f32 PSUM.  Measured end-to-end error vs the f32
reference is ~8e-4 (budget 2e-2).  Local terms (H, Za, Zb, weights)
stay f32.

SpMM on-device: edges are bucketed host-side by (dst-chunk of 128,
src-window of 25000 [int16 gather addressing]) into a static, padded
schedule shared by all 8 cores (one SPMD program).  Messages are fetched
with dma_gather (512B f32 rows; <=1024 idxs per call — larger wedges the
device), segment-summed per dst-chunk with one-hot matmuls accumulating
in PSUM (one-hots built on the DVE from per-edge slot ids vs an iota;
padded slots get slot=255 and multiply to zero), then scaled by
dinv[dst] and consumed.  Za/Zb spill to DRAM to keep the f32 message
tiles resident in SBUF.
"""

import numpy as np
import ml_dtypes

import concourse.bass as bass
import concourse.bacc as bacc
import concourse.mybir as mybir
import concourse.tile as tile
import concourse.bass_utils as bass_utils

P = 128
F32 = mybir.dt.float32
BF16 = mybir.dt.bfloat16
I16 = mybir.dt.int16


class Cfg:
    def __init__(self, N, ncores, D, OUT, num_hid, wsz, G):
        self.N = N
        self.NCORES = ncores
        self.SHARD = N // ncores
        self.NCHUNK = (self.SHARD + P - 1) // P
        self.NPAD = self.NCHUNK * P          # padded shard rows
        self.D = D                            # hidden width (=IN)
        self.OUT = OUT
        self.NL = num_hid + 2                 # total ChebConv layers
        self.WSZ = wsz                        # src window size (int16 range)
        self.NW = (N + wsz - 1) // wsz
        self.G = G                            # chunks per gather group
        assert self.NCHUNK % G == 0
        self.NG = self.NCHUNK // G
        # CAPW filled in by prep (data dependent, 128-aligned)
        self.CAPW = None
        self.NBW = None                       # blocks per (chunk, window)
        self.NB = None                        # blocks per chunk
        self.RW = None                        # slots per gather call
        self.RB = None                        # blocks per gather call


def make_cfg_full():
    return Cfg(N=100000, ncores=8, D=128, OUT=40, num_hid=5, wsz=25000, G=7)


def prep(inputs, cfg):
    """Host-side graph preprocessing -> per-core input maps."""
    src = np.asarray(inputs["src"]).astype(np.int64)
    dst = np.asarray(inputs["dst"]).astype(np.int64)
    feat = np.asarray(inputs["features"], dtype=np.float32)
    N, C = cfg.N, cfg.NCORES

    deg = np.bincount(dst, minlength=N).astype(np.float32)
    dinv = np.clip(deg, 1.0, None) ** -0.5

    core = dst // cfg.SHARD
    loc = dst % cfg.SHARD                   # row within the owning shard
    chunk = loc // P
    lane = loc % P                          # slot id within chunk
    win = src // cfg.WSZ
    idx16 = (src % cfg.WSZ).astype(np.int16)

    # per (core, chunk, window) edge lists
    key = ((core * cfg.NCHUNK + chunk) * cfg.NW + win).astype(np.int64)
    order = np.argsort(key, kind="stable")
    counts = np.bincount(key, minlength=C * cfg.NCHUNK * cfg.NW)
    counts = counts.reshape(C, cfg.NCHUNK, cfg.NW)
    capw = int(counts.max())
    cfg.CAPW = ((capw + P - 1) // P) * P
    cfg.NBW = cfg.CAPW // P
    cfg.NB = cfg.NBW * cfg.NW
    cfg.RW = cfg.G * cfg.CAPW
    cfg.RB = cfg.RW // P

    src_sorted = idx16[order]
    lane_sorted = lane[order].astype(np.int32)
    starts = np.zeros(C * cfg.NCHUNK * cfg.NW + 1, np.int64)
    np.cumsum(counts.ravel(), out=starts[1:])

    # layer weights -> [NL, D, 3*fout] slabs (fp32)
    NL = cfg.NL
    wabc = np.zeros((NL, cfg.D, 3 * cfg.D), np.float32)
    bbc = np.zeros((NL, P, cfg.D), np.float32)

    def pack(Wfull, b, li, fout):
        Wfull = np.asarray(Wfull, dtype=np.float32)
        d = Wfull.shape[0] // 3
        W0, W1, W2 = Wfull[:d], Wfull[d:2 * d], Wfull[2 * d:]
        Wa, Wb, Wc = W0 - W2, -W1, 2.0 * W2
        wabc[li, :, 0 * fout:1 * fout] = Wa
        wabc[li, :, 1 * fout:2 * fout] = Wb
        wabc[li, :, 2 * fout:3 * fout] = Wc
        bbc[li, :, :fout] = np.tile(np.asarray(b, dtype=np.float32)[None, :], (P, 1))

    pack(inputs["W0"], inputs["b0"], 0, cfg.D)
    for i in range(NL - 2):
        pack(np.asarray(inputs["Wh"])[i], np.asarray(inputs["bh"])[i], 1 + i, cfg.D)
    pack(inputs["Wl"], inputs["bl"], NL - 1, cfg.OUT)

    iota_rep = np.tile(
        np.arange(P, dtype=np.float32)[None, None, :], (P, cfg.NB, 1))
    ident = np.eye(P, dtype=np.float32)

    in_maps = []
    for c in range(C):
        tot = cfg.NCHUNK * cfg.NB * P        # slots per spmm
        idxs = np.zeros(tot, np.int16)
        slots_cols = np.full((P, cfg.NCHUNK * cfg.NB), 255.0, np.float32)
        pos = 0
        for g in range(cfg.NG):
            for w in range(cfg.NW):
                for i in range(cfg.G):
                    q = g * cfg.G + i
                    k = (c * cfg.NCHUNK + q) * cfg.NW + w
                    s, e = starts[k], starts[k + 1]
                    n = e - s
                    idxs[pos:pos + n] = src_sorted[s:e]
                    # block-columns for this (q, w): cols q*NB + w*NBW + j
                    seg_sl = lane_sorted[s:e]
                    for j in range(cfg.NBW):
                        col = q * cfg.NB + w * cfg.NBW + j
                        a, b_ = j * P, min((j + 1) * P, n)
                        if a < n:
                            slots_cols[:b_ - a, col] = seg_sl[a:b_]
                    pos += cfg.CAPW
        assert pos == tot
        # wrapped idx layout [128, tot/16]
        wr = idxs.reshape(tot // 16, 16).T
        idxs_w = np.tile(wr, (8, 1)).copy()

        sh0 = c * cfg.SHARD
        fpad = np.zeros((cfg.NPAD, cfg.D), np.float32)
        fpad[:cfg.SHARD] = feat[sh0:sh0 + cfg.SHARD]
        dv = dinv[sh0:sh0 + cfg.SHARD]
        dm = np.ones((P, cfg.NCHUNK), np.float32)
        for q in range(cfg.NCHUNK):
            r = min(P, cfg.SHARD - q * P)
            dm[:r, q] = dv[q * P:q * P + r]
        in_maps.append(dict(
            feat=fpad,
            idxs=idxs_w,
            slots=slots_cols,
            dinvc=dm,
            dinv2c=(dm * dm).astype(np.float32),
            wabc=wabc,
            bbc=bbc,
            iotarep=iota_rep,
            ident=ident,
        ))
    return in_maps


def build(nc, cfg):
    NL, D, OUT = cfg.NL, cfg.D, cfg.OUT
    NCH, NB, NBW, NW, G, NG = cfg.NCHUNK, cfg.NB, cfg.NBW, cfg.NW, cfg.G, cfg.NG
    RW, RB = cfg.RW, cfg.RB
    TOT16 = NCH * NB * P // 16

    feat_in = nc.dram_tensor("feat", [cfg.NPAD, D], F32, kind="ExternalInput")
    idxs_in = nc.dram_tensor("idxs", [P, TOT16], I16, kind="ExternalInput")
    slots_in = nc.dram_tensor("slots", [P, NCH * NB], F32, kind="ExternalInput")
    dinv_in = nc.dram_tensor("dinvc", [P, NCH], F32, kind="ExternalInput")
    dinv2_in = nc.dram_tensor("dinv2c", [P, NCH], F32, kind="ExternalInput")
    wabc_in = nc.dram_tensor("wabc", [NL, D, 3 * D], F32, kind="ExternalInput")
    bbc_in = nc.dram_tensor("bbc", [NL, P, D], F32, kind="ExternalInput")
    iota_in = nc.dram_tensor("iotarep", [P, NB, P], F32, kind="ExternalInput")
    ident_in = nc.dram_tensor("ident", [P, P], F32, kind="ExternalInput")
    out_dram = nc.dram_tensor("out", [cfg.SHARD, OUT], F32, kind="ExternalOutput")

    with tile.TileContext(nc) as tc:
        with (
            tc.tile_pool(name="persist", bufs=1) as pp,
            tc.tile_pool(name="work", bufs=2) as wk,
            tc.tile_pool(name="msgp", bufs=5) as mp,
            tc.tile_pool(name="psum", bufs=2, space="PSUM") as ps,
            tc.tile_pool(name="praw", bufs=4, space="PSUM") as pr,
            tc.tile_pool(name="dram", bufs=1, space="DRAM") as dr,
        ):
            # persistent state (Za/Zb spill to DRAM; SBUF holds H + msg tiles)
            H = pp.tile([P, NCH, D], F32, tag="H")
            iota_t = pp.tile([P, NB, P], F32, tag="iota")
            ident_t = pp.tile([P, P], F32, tag="ident")
            dinv_t = pp.tile([P, NCH], F32, tag="dinv")
            dinv2_t = pp.tile([P, NCH], F32, tag="dinv2")
            idx_t = pp.tile([P, TOT16], I16, tag="idxs")
            slots_t = pp.tile([P, NCH * NB], F32, tag="slots")
            nc.sync.dma_start(iota_t[:], iota_in[:, :, :])
            nc.sync.dma_start(ident_t[:], ident_in[:, :])
            nc.sync.dma_start(dinv_t[:], dinv_in[:, :])
            nc.sync.dma_start(dinv2_t[:], dinv2_in[:, :])
            nc.sync.dma_start(idx_t[:], idxs_in[:, :])
            nc.sync.dma_start(slots_t[:], slots_in[:, :])
            nc.sync.dma_start(
                H[:], feat_in[:, :].rearrange("(q p) f -> p q f", p=P))

            gin1 = dr.tile([cfg.SHARD, D], BF16, tag="gin1")
            gout1 = dr.tile([cfg.N, D], BF16, tag="gout1")
            gin2 = dr.tile([cfg.SHARD, D], BF16, tag="gin2")
            gout2 = dr.tile([cfg.N, D], BF16, tag="gout2")
            za_dram = dr.tile([cfg.NPAD, D], F32, tag="za")
            zb_dram = dr.tile([cfg.NPAD, D], F32, tag="zb")

            def ag(gin, gout):
                nc.gpsimd.collective_compute(
                    "AllGather",
                    mybir.AluOpType.bypass,
                    replica_groups=[list(range(cfg.NCORES))],
                    ins=[gin.opt()],
                    outs=[gout.opt()],
                )

            def spmm(gout, fout, consume):
                """Gather from gout, segment-sum per chunk, call consume(q, praw_ap)."""
                # dma_gather wedges the device above ~1k idxs/call: sub-gather
                # per chunk (CAPW slots) out of the persistent idx tile.
                assert cfg.CAPW <= 1024
                for g in range(NG):
                    mg = []
                    for w in range(NW):
                        r = g * NW + w
                        m = mp.tile([P, RB, D], BF16, tag="msg")
                        c16 = cfg.CAPW // 16
                        r16 = RW // 16
                        for i in range(G):
                            nc.gpsimd.dma_gather(
                                m[:, i * NBW:(i + 1) * NBW, :],
                                gout[w * cfg.WSZ:min((w + 1) * cfg.WSZ, cfg.N), :],
                                idx_t[:, r * r16 + i * c16:r * r16 + (i + 1) * c16],
                                cfg.CAPW,
                                cfg.CAPW,
                                D,
                            )
                        mg.append(m)
                    for i in range(G):
                        q = g * G + i
                        oh = wk.tile([P, NB, P], BF16, tag="oh")
                        nc.vector.tensor_tensor(
                            out=oh[:],
                            in0=iota_t[:],
                            in1=slots_t[:, q * NB:(q + 1) * NB].to_broadcast(
                                [P, NB, P]),
                            op=mybir.AluOpType.is_equal,
                        )
                        acc = pr.tile([P, D], F32, tag="praw")
                        nb = 0
                        for w in range(NW):
                            for j in range(NBW):
                                nc.tensor.matmul(
                                    acc[:, :fout],
                                    lhsT=oh[:, w * NBW + j, :],
                                    rhs=mg[w][:, i * NBW + j, :fout],
                                    start=(nb == 0),
                                    stop=(nb == NB - 1),
                                )
                                nb += 1
                        consume(q, acc)

            for li in range(NL):
                fout = OUT if li == NL - 1 else D
                wt = wk.tile([P, 3 * D], F32, tag="wt")
                nc.sync.dma_start(wt[:], wabc_in[li, :, :])
                bb = wk.tile([P, D], F32, tag="bbc")
                nc.sync.dma_start(bb[:], bbc_in[li, :, :])

                # Z phase: Za' = H Wa + b; ZbD = dinv*(H Wb); U2 = dinv*(H Wc) -> gin1
                for q in range(NCH):
                    rows = min(P, cfg.SHARD - q * P)
                    tp = ps.tile([P, P], F32, tag="tp")
                    nc.tensor.transpose(tp[:], H[:, q, :], ident_t[:])
                    ht = wk.tile([P, P], F32, tag="ht")
                    nc.vector.tensor_copy(ht[:], tp[:])
                    pz = ps.tile([P, 3 * D], F32, tag="pz")
                    nc.tensor.matmul(pz[:, :3 * fout], lhsT=ht[:], rhs=wt[:, :3 * fout],
                                     start=True, stop=True)
                    za_t = wk.tile([P, P], F32, tag="za_t")
                    nc.vector.tensor_tensor(
                        out=za_t[:, :fout], in0=pz[:, 0:fout], in1=bb[:, :fout],
                        op=mybir.AluOpType.add)
                    nc.sync.dma_start(za_dram[q * P:(q + 1) * P, :fout],
                                      za_t[:, :fout])
                    zb_t = wk.tile([P, P], F32, tag="zb_t")
                    nc.vector.tensor_scalar(
                        out=zb_t[:, :fout], in0=pz[:, fout:2 * fout],
                        scalar1=dinv_t[:, q:q + 1], scalar2=None,
                        op0=mybir.AluOpType.mult)
                    nc.sync.dma_start(zb_dram[q * P:(q + 1) * P, :fout],
                                      zb_t[:, :fout])
                    u2 = wk.tile([P, P], BF16, tag="u2")
                    nc.vector.tensor_scalar(
                        out=u2[:, :fout], in0=pz[:, 2 * fout:3 * fout],
                        scalar1=dinv_t[:, q:q + 1], scalar2=None,
                        op0=mybir.AluOpType.mult)
                    nc.sync.dma_start(gin1[q * P:q * P + rows, :fout],
                                      u2[:rows, :fout])

                ag(gin1, gout1)

                def consume1(q, acc, fout=fout):
                    rows = min(P, cfg.SHARD - q * P)
                    zb_l = wk.tile([P, P], F32, tag="zb_l")
                    nc.sync.dma_start(zb_l[:, :fout],
                                      zb_dram[q * P:(q + 1) * P, :fout])
                    t1 = wk.tile([P, P], F32, tag="t1")
                    nc.vector.tensor_scalar(
                        out=t1[:, :fout], in0=acc[:, :fout],
                        scalar1=dinv2_t[:, q:q + 1], scalar2=None,
                        op0=mybir.AluOpType.mult)
                    t2 = wk.tile([P, P], BF16, tag="t2")
                    nc.vector.tensor_tensor(
                        out=t2[:, :fout], in0=t1[:, :fout], in1=zb_l[:, :fout],
                        op=mybir.AluOpType.add)
                    nc.sync.dma_start(gin2[q * P:q * P + rows, :fout],
                                      t2[:rows, :fout])

                spmm(gout1, fout, consume1)
                ag(gin2, gout2)

                if li < NL - 1:
                    def consume2(q, acc, fout=fout):
                        za_l = wk.tile([P, P], F32, tag="za_l")
                        nc.sync.dma_start(za_l[:, :fout],
                                          za_dram[q * P:(q + 1) * P, :fout])
                        t1 = wk.tile([P, P], F32, tag="t1")
                        nc.vector.tensor_scalar(
                            out=t1[:, :fout], in0=acc[:, :fout],
                            scalar1=dinv_t[:, q:q + 1], scalar2=None,
                            op0=mybir.AluOpType.mult)
                        t3 = wk.tile([P, P], F32, tag="t3")
                        nc.vector.tensor_tensor(
                            out=t3[:, :fout], in0=t1[:, :fout], in1=za_l[:, :fout],
                            op=mybir.AluOpType.add)
                        nc.scalar.activation(
                            H[:, q, :fout], t3[:, :fout],
                            mybir.ActivationFunctionType.Relu)
                else:
                    def consume2(q, acc, fout=fout):
                        rows = min(P, cfg.SHARD - q * P)
                        za_l = wk.tile([P, P], F32, tag="za_l")
                        nc.sync.dma_start(za_l[:, :fout],
                                          za_dram[q * P:(q + 1) * P, :fout])
                        t1 = wk.tile([P, P], F32, tag="t1")
                        nc.vector.tensor_scalar(
                            out=t1[:, :fout], in0=acc[:, :fout],
                            scalar1=dinv_t[:, q:q + 1], scalar2=None,
                            op0=mybir.AluOpType.mult)
                        t3 = wk.tile([P, P], F32, tag="t3")
                        nc.vector.tensor_tensor(
                            out=t3[:, :fout], in0=t1[:, :fout], in1=za_l[:, :fout],
                            op=mybir.AluOpType.add)
                        ho = wk.tile([P, P], F32, tag="ho")
                        nc.scalar.activation(
                            ho[:, :fout], t3[:, :fout],
                            mybir.ActivationFunctionType.Relu)
                        nc.sync.dma_start(out_dram[q * P:q * P + rows, :fout],
                                          ho[:rows, :fout])

                spmm(gout2, fout, consume2)
    return nc


def run(inputs, cfg, trace=False):
    in_maps = prep(inputs, cfg)
    nc = bacc.Bacc("TRN2", target_bir_lowering=False, debug=False,
                   num_devices=cfg.NCORES)
    build(nc, cfg)
    nc.compile()
    res = bass_utils.run_bass_kernel_spmd(
        nc, in_maps, core_ids=list(range(cfg.NCORES)), trace=trace)
    out = np.concatenate([res.results[c]["out"] for c in range(cfg.NCORES)],
                         axis=0)
    return out[:cfg.N], res


def kernel(**inputs) -> np.ndarray:
    cfg = make_cfg_full()
    out, _ = run(inputs, cfg)
    return out.astype(np.float32)



# revision 9
# speedup vs baseline: 1.0571x; 1.0285x over previous
"""ChebNet (K=3, 7 ChebConv layers) on 8 Trainium2 NeuronCores.

Strategy
--------
Nodes are partitioned contiguously across the 8 cores (12500/core); each
core owns the edges whose dst falls in its shard.  Each ChebConv layer
    relu(cat(X0,X1,X2) @ W + b),  X1 = -A_hat X,  X2 = -2 A_hat X1 - X0
is rewritten (exactly) as
    relu(H @ Wa + A_hat (H @ Wb + A_hat (H @ Wc)) + b)
with Wa = W0' - W2', Wb = -W1', Wc = 2 W2'  (W = [W0'; W1'; W2']),
so the SpMM operates on post-matmul activations and the layer needs
exactly two halo exchanges (AllGather of the f32, dinv-prescaled shard).
The message path (AllGather payload + gathered messages) is bf16 with
f32 PSUM accumulation: the payload is rounded to bf16 once per
exchange, and the one-hot segment-sum matmul accumulates those bf16
messages exactly in f32 PSUM.  Measured end-to-end error vs the f32
reference is ~8e-4 (budget 2e-2).  Local terms (H, Za, Zb, weights)
stay f32.

SpMM on-device: edges are bucketed host-side by (dst-chunk of 128,
src-window of 25000 [int16 gather addressing]) into a static, padded
schedule shared by all 8 cores (one SPMD program).  Messages are fetched
with dma_gather (512B f32 rows; <=1024 idxs per call — larger wedges the
device), segment-summed per dst-chunk with one-hot matmuls accumulating
in PSUM (one-hots built on the DVE from per-edge slot ids vs an iota;
padded slots get slot=255 and multiply to zero), then scaled by
dinv[dst] and consumed.  Za/Zb spill to DRAM to keep the f32 message
tiles resident in SBUF.
"""

import numpy as np
import ml_dtypes

import concourse.bass as bass
import concourse.bacc as bacc
import concourse.mybir as mybir
import concourse.tile as tile
import concourse.bass_utils as bass_utils

P = 128
F32 = mybir.dt.float32
BF16 = mybir.dt.bfloat16
I16 = mybir.dt.int16


class Cfg:
    def __init__(self, N, ncores, D, OUT, num_hid, wsz, G):
        self.N = N
        self.NCORES = ncores
        self.SHARD = N // ncores
        self.NCHUNK = (self.SHARD + P - 1) // P
        self.NPAD = self.NCHUNK * P          # padded shard rows
        self.D = D                            # hidden width (=IN)
        self.OUT = OUT
        self.NL = num_hid + 2                 # total ChebConv layers
        self.WSZ = wsz                        # src window size (int16 range)
        self.NW = (N + wsz - 1) // wsz
        self.G = G                            # chunks per gather group
        assert self.NCHUNK % G == 0
        self.NG = self.NCHUNK // G
        # CAPW filled in by prep (data dependent, 128-aligned)
        self.CAPW = None
        self.NBW = None                       # blocks per (chunk, window)
        self.NB = None                        # blocks per chunk
        self.RW = None                        # slots per gather call
        self.RB = None                        # blocks per gather call


def make_cfg_full():
    return Cfg(N=100000, ncores=8, D=128, OUT=40, num_hid=5, wsz=25000, G=7)


def prep(inputs, cfg):
    """Host-side graph preprocessing -> per-core input maps."""
    src = np.asarray(inputs["src"]).astype(np.int64)
    dst = np.asarray(inputs["dst"]).astype(np.int64)
    feat = np.asarray(inputs["features"], dtype=np.float32)
    N, C = cfg.N, cfg.NCORES

    deg = np.bincount(dst, minlength=N).astype(np.float32)
    dinv = np.clip(deg, 1.0, None) ** -0.5

    core = dst // cfg.SHARD
    loc = dst % cfg.SHARD                   # row within the owning shard
    chunk = loc // P
    lane = loc % P                          # slot id within chunk
    win = src // cfg.WSZ
    idx16 = (src % cfg.WSZ).astype(np.int16)

    # per (core, chunk, window) edge lists
    key = ((core * cfg.NCHUNK + chunk) * cfg.NW + win).astype(np.int64)
    order = np.argsort(key, kind="stable")
    counts = np.bincount(key, minlength=C * cfg.NCHUNK * cfg.NW)
    counts = counts.reshape(C, cfg.NCHUNK, cfg.NW)
    capw = int(counts.max())
    cfg.CAPW = ((capw + P - 1) // P) * P
    cfg.NBW = cfg.CAPW // P
    cfg.NB = cfg.NBW * cfg.NW
    cfg.RW = cfg.G * cfg.CAPW
    cfg.RB = cfg.RW // P

    src_sorted = idx16[order]
    lane_sorted = lane[order].astype(np.int32)
    starts = np.zeros(C * cfg.NCHUNK * cfg.NW + 1, np.int64)
    np.cumsum(counts.ravel(), out=starts[1:])

    # layer weights -> [NL, D, 3*fout] slabs (fp32)
    NL = cfg.NL
    wabc = np.zeros((NL, cfg.D, 3 * cfg.D), np.float32)
    bbc = np.zeros((NL, P, cfg.D), np.float32)

    def pack(Wfull, b, li, fout):
        Wfull = np.asarray(Wfull, dtype=np.float32)
        d = Wfull.shape[0] // 3
        W0, W1, W2 = Wfull[:d], Wfull[d:2 * d], Wfull[2 * d:]
        Wa, Wb, Wc = W0 - W2, -W1, 2.0 * W2
        wabc[li, :, 0 * fout:1 * fout] = Wa
        wabc[li, :, 1 * fout:2 * fout] = Wb
        wabc[li, :, 2 * fout:3 * fout] = Wc
        bbc[li, :, :fout] = np.tile(np.asarray(b, dtype=np.float32)[None, :], (P, 1))

    pack(inputs["W0"], inputs["b0"], 0, cfg.D)
    for i in range(NL - 2):
        pack(np.asarray(inputs["Wh"])[i], np.asarray(inputs["bh"])[i], 1 + i, cfg.D)
    pack(inputs["Wl"], inputs["bl"], NL - 1, cfg.OUT)

    iota_rep = np.tile(
        np.arange(P, dtype=np.float32)[None, None, :], (P, cfg.NB, 1))
    ident = np.eye(P, dtype=np.float32)

    in_maps = []
    for c in range(C):
        tot = cfg.NCHUNK * cfg.NB * P        # slots per spmm
        idxs = np.zeros(tot, np.int16)
        slots_cols = np.full((P, cfg.NCHUNK * cfg.NB), 255.0, np.float32)
        pos = 0
        for g in range(cfg.NG):
            for w in range(cfg.NW):
                for i in range(cfg.G):
                    q = g * cfg.G + i
                    k = (c * cfg.NCHUNK + q) * cfg.NW + w
                    s, e = starts[k], starts[k + 1]
                    n = e - s
                    idxs[pos:pos + n] = src_sorted[s:e]
                    # block-columns for this (q, w): cols q*NB + w*NBW + j
                    seg_sl = lane_sorted[s:e]
                    for j in range(cfg.NBW):
                        col = q * cfg.NB + w * cfg.NBW + j
                        a, b_ = j * P, min((j + 1) * P, n)
                        if a < n:
                            slots_cols[:b_ - a, col] = seg_sl[a:b_]
                    pos += cfg.CAPW
        assert pos == tot
        # wrapped idx layout [128, tot/16]
        wr = idxs.reshape(tot // 16, 16).T
        idxs_w = np.tile(wr, (8, 1)).copy()

        sh0 = c * cfg.SHARD
        fpad = np.zeros((cfg.NPAD, cfg.D), np.float32)
        fpad[:cfg.SHARD] = feat[sh0:sh0 + cfg.SHARD]
        dv = dinv[sh0:sh0 + cfg.SHARD]
        dm = np.ones((P, cfg.NCHUNK), np.float32)
        for q in range(cfg.NCHUNK):
            r = min(P, cfg.SHARD - q * P)
            dm[:r, q] = dv[q * P:q * P + r]
        in_maps.append(dict(
            feat=fpad,
            idxs=idxs_w,
            slots=slots_cols,
            dinvc=dm,
            dinv2c=(dm * dm).astype(np.float32),
            wabc=wabc,
            bbc=bbc,
            iotarep=iota_rep,
            ident=ident,
        ))
    return in_maps


def build(nc, cfg):
    NL, D, OUT = cfg.NL, cfg.D, cfg.OUT
    NCH, NB, NBW, NW, G, NG = cfg.NCHUNK, cfg.NB, cfg.NBW, cfg.NW, cfg.G, cfg.NG
    RW, RB = cfg.RW, cfg.RB
    TOT16 = NCH * NB * P // 16

    feat_in = nc.dram_tensor("feat", [cfg.NPAD, D], F32, kind="ExternalInput")
    idxs_in = nc.dram_tensor("idxs", [P, TOT16], I16, kind="ExternalInput")
    slots_in = nc.dram_tensor("slots", [P, NCH * NB], F32, kind="ExternalInput")
    dinv_in = nc.dram_tensor("dinvc", [P, NCH], F32, kind="ExternalInput")
    dinv2_in = nc.dram_tensor("dinv2c", [P, NCH], F32, kind="ExternalInput")
    wabc_in = nc.dram_tensor("wabc", [NL, D, 3 * D], F32, kind="ExternalInput")
    bbc_in = nc.dram_tensor("bbc", [NL, P, D], F32, kind="ExternalInput")
    iota_in = nc.dram_tensor("iotarep", [P, NB, P], F32, kind="ExternalInput")
    ident_in = nc.dram_tensor("ident", [P, P], F32, kind="ExternalInput")
    out_dram = nc.dram_tensor("out", [cfg.SHARD, OUT], F32, kind="ExternalOutput")

    with tile.TileContext(nc) as tc:
        with (
            tc.tile_pool(name="persist", bufs=1) as pp,
            tc.tile_pool(name="work", bufs=2) as wk,
            tc.tile_pool(name="msgp", bufs=5) as mp,
            tc.tile_pool(name="psum", bufs=2, space="PSUM") as ps,
            tc.tile_pool(name="praw", bufs=4, space="PSUM") as pr,
            tc.tile_pool(name="dram", bufs=1, space="DRAM") as dr,
        ):
            # persistent state (Za/Zb spill to DRAM; SBUF holds H + msg tiles)
            H = pp.tile([P, NCH, D], F32, tag="H")
            iota_t = pp.tile([P, NB, P], F32, tag="iota")
            ident_t = pp.tile([P, P], F32, tag="ident")
            dinv_t = pp.tile([P, NCH], F32, tag="dinv")
            dinv2_t = pp.tile([P, NCH], F32, tag="dinv2")
            idx_t = pp.tile([P, TOT16], I16, tag="idxs")
            slots_t = pp.tile([P, NCH * NB], F32, tag="slots")
            nc.sync.dma_start(iota_t[:], iota_in[:, :, :])
            nc.sync.dma_start(ident_t[:], ident_in[:, :])
            nc.sync.dma_start(dinv_t[:], dinv_in[:, :])
            nc.sync.dma_start(dinv2_t[:], dinv2_in[:, :])
            nc.sync.dma_start(idx_t[:], idxs_in[:, :])
            nc.sync.dma_start(slots_t[:], slots_in[:, :])
            nc.sync.dma_start(
                H[:], feat_in[:, :].rearrange("(q p) f -> p q f", p=P))

            gin1 = dr.tile([cfg.SHARD, D], BF16, tag="gin1")
            gin2 = dr.tile([cfg.SHARD, D], BF16, tag="gin2")
            # Shared DRAM tensors may be written by exactly one instruction,
            # so each AllGather gets its own output buffer.
            gout1s = [
                dr.tile([cfg.N, D], BF16, tag=f"gout1_l{li}", name=f"gout1_l{li}",
                        addr_space="Shared")
                for li in range(NL)
            ]
            gout2s = [
                dr.tile([cfg.N, D], BF16, tag=f"gout2_l{li}", name=f"gout2_l{li}",
                        addr_space="Shared")
                for li in range(NL)
            ]
            za_dram = dr.tile([cfg.NPAD, D], F32, tag="za")
            zb_dram = dr.tile([cfg.NPAD, D], F32, tag="zb")

            def ag(gin, gout):
                nc.gpsimd.collective_compute(
                    "AllGather",
                    mybir.AluOpType.bypass,
                    replica_groups=[list(range(cfg.NCORES))],
                    ins=[gin.opt()],
                    outs=[gout.opt()],
                )

            def spmm(gout, fout, consume):
                """Gather from gout, segment-sum per chunk, call consume(q, praw_ap)."""
                # dma_gather wedges the device above ~1k idxs/call: sub-gather
                # per chunk (CAPW slots) out of the persistent idx tile.
                assert cfg.CAPW <= 1024
                for g in range(NG):
                    mg = []
                    for w in range(NW):
                        r = g * NW + w
                        m = mp.tile([P, RB, D], BF16, tag="msg")
                        c16 = cfg.CAPW // 16
                        r16 = RW // 16
                        for i in range(G):
                            nc.gpsimd.dma_gather(
                                m[:, i * NBW:(i + 1) * NBW, :],
                                gout[w * cfg.WSZ:min((w + 1) * cfg.WSZ, cfg.N), :],
                                idx_t[:, r * r16 + i * c16:r * r16 + (i + 1) * c16],
                                cfg.CAPW,
                                cfg.CAPW,
                                D,
                            )
                        mg.append(m)
                    for i in range(G):
                        q = g * G + i
                        oh = wk.tile([P, NB, P], BF16, tag="oh")
                        nc.vector.tensor_tensor(
                            out=oh[:],
                            in0=iota_t[:],
                            in1=slots_t[:, q * NB:(q + 1) * NB].to_broadcast(
                                [P, NB, P]),
                            op=mybir.AluOpType.is_equal,
                        )
                        acc = pr.tile([P, D], F32, tag="praw")
                        nb = 0
                        for w in range(NW):
                            for j in range(NBW):
                                nc.tensor.matmul(
                                    acc[:, :fout],
                                    lhsT=oh[:, w * NBW + j, :],
                                    rhs=mg[w][:, i * NBW + j, :fout],
                                    start=(nb == 0),
                                    stop=(nb == NB - 1),
                                )
                                nb += 1
                        consume(q, acc)

            for li in range(NL):
                fout = OUT if li == NL - 1 else D
                wt = wk.tile([P, 3 * D], F32, tag="wt")
                nc.sync.dma_start(wt[:], wabc_in[li, :, :])
                bb = wk.tile([P, D], F32, tag="bbc")
                nc.sync.dma_start(bb[:], bbc_in[li, :, :])

                # Z phase: Za' = H Wa + b; ZbD = dinv*(H Wb); U2 = dinv*(H Wc) -> gin1
                for q in range(NCH):
                    rows = min(P, cfg.SHARD - q * P)
                    tp = ps.tile([P, P], F32, tag="tp")
                    nc.tensor.transpose(tp[:], H[:, q, :], ident_t[:])
                    ht = wk.tile([P, P], F32, tag="ht")
                    nc.vector.tensor_copy(ht[:], tp[:])
                    pz = ps.tile([P, 3 * D], F32, tag="pz")
                    nc.tensor.matmul(pz[:, :3 * fout], lhsT=ht[:], rhs=wt[:, :3 * fout],
                                     start=True, stop=True)
                    za_t = wk.tile([P, P], F32, tag="za_t")
                    nc.vector.tensor_tensor(
                        out=za_t[:, :fout], in0=pz[:, 0:fout], in1=bb[:, :fout],
                        op=mybir.AluOpType.add)
                    nc.sync.dma_start(za_dram[q * P:(q + 1) * P, :fout],
                                      za_t[:, :fout])
                    zb_t = wk.tile([P, P], F32, tag="zb_t")
                    nc.vector.tensor_scalar(
                        out=zb_t[:, :fout], in0=pz[:, fout:2 * fout],
                        scalar1=dinv_t[:, q:q + 1], scalar2=None,
                        op0=mybir.AluOpType.mult)
                    nc.sync.dma_start(zb_dram[q * P:(q + 1) * P, :fout],
                                      zb_t[:, :fout])
                    u2 = wk.tile([P, P], BF16, tag="u2")
                    nc.vector.tensor_scalar(
                        out=u2[:, :fout], in0=pz[:, 2 * fout:3 * fout],
                        scalar1=dinv_t[:, q:q + 1], scalar2=None,
                        op0=mybir.AluOpType.mult)
                    nc.sync.dma_start(gin1[q * P:q * P + rows, :fout],
                                      u2[:rows, :fout])

                ag(gin1, gout1s[li])

                def consume1(q, acc, fout=fout):
                    rows = min(P, cfg.SHARD - q * P)
                    zb_l = wk.tile([P, P], F32, tag="zb_l")
                    nc.sync.dma_start(zb_l[:, :fout],
                                      zb_dram[q * P:(q + 1) * P, :fout])
                    t1 = wk.tile([P, P], F32, tag="t1")
                    nc.vector.tensor_scalar(
                        out=t1[:, :fout], in0=acc[:, :fout],
                        scalar1=dinv2_t[:, q:q + 1], scalar2=None,
                        op0=mybir.AluOpType.mult)
                    t2 = wk.tile([P, P], BF16, tag="t2")
                    nc.vector.tensor_tensor(
                        out=t2[:, :fout], in0=t1[:, :fout], in1=zb_l[:, :fout],
                        op=mybir.AluOpType.add)
                    nc.sync.dma_start(gin2[q * P:q * P + rows, :fout],
                                      t2[:rows, :fout])

                spmm(gout1s[li], fout, consume1)
                ag(gin2, gout2s[li])

                if li < NL - 1:
                    def consume2(q, acc, fout=fout):
                        za_l = wk.tile([P, P], F32, tag="za_l")
                        nc.sync.dma_start(za_l[:, :fout],
                                          za_dram[q * P:(q + 1) * P, :fout])
                        t1 = wk.tile([P, P], F32, tag="t1")
                        nc.vector.tensor_scalar(
                            out=t1[:, :fout], in0=acc[:, :fout],
                            scalar1=dinv_t[:, q:q + 1], scalar2=None,
                            op0=mybir.AluOpType.mult)
                        t3 = wk.tile([P, P], F32, tag="t3")
                        nc.vector.tensor_tensor(
                            out=t3[:, :fout], in0=t1[:, :fout], in1=za_l[:, :fout],
                            op=mybir.AluOpType.add)
                        nc.scalar.activation(
                            H[:, q, :fout], t3[:, :fout],
                            mybir.ActivationFunctionType.Relu)
                else:
                    def consume2(q, acc, fout=fout):
                        rows = min(P, cfg.SHARD - q * P)
                        za_l = wk.tile([P, P], F32, tag="za_l")
                        nc.sync.dma_start(za_l[:, :fout],
                                          za_dram[q * P:(q + 1) * P, :fout])
                        t1 = wk.tile([P, P], F32, tag="t1")
                        nc.vector.tensor_scalar(
                            out=t1[:, :fout], in0=acc[:, :fout],
                            scalar1=dinv_t[:, q:q + 1], scalar2=None,
                            op0=mybir.AluOpType.mult)
                        t3 = wk.tile([P, P], F32, tag="t3")
                        nc.vector.tensor_tensor(
                            out=t3[:, :fout], in0=t1[:, :fout], in1=za_l[:, :fout],
                            op=mybir.AluOpType.add)
                        ho = wk.tile([P, P], F32, tag="ho")
                        nc.scalar.activation(
                            ho[:, :fout], t3[:, :fout],
                            mybir.ActivationFunctionType.Relu)
                        nc.sync.dma_start(out_dram[q * P:q * P + rows, :fout],
                                          ho[:rows, :fout])

                spmm(gout2s[li], fout, consume2)
    return nc


def run(inputs, cfg, trace=False):
    in_maps = prep(inputs, cfg)
    nc = bacc.Bacc("TRN2", target_bir_lowering=False, debug=False,
                   num_devices=cfg.NCORES)
    build(nc, cfg)
    nc.compile()
    res = bass_utils.run_bass_kernel_spmd(
        nc, in_maps, core_ids=list(range(cfg.NCORES)), trace=trace)
    out = np.concatenate([res.results[c]["out"] for c in range(cfg.NCORES)],
                         axis=0)
    return out[:cfg.N], res


def kernel(**inputs) -> np.ndarray:
    cfg = make_cfg_full()
    out, _ = run(inputs, cfg)
    return out.astype(np.float32)

